# revision 16
# baseline (speedup 1.0000x reference)
"""CLIP attention (B=8, S=1024, H=1024, 16 heads) on 8 TRN2 NeuronCores.

Sharding: data-parallel over batch — core b computes attention for x[b].

Per-core dataflow (projections in float32r; attention operands in bf16):
  phase 0: x -> x^T via PE transposes                      (x^T[h, s])
  phase 1: V = x^T-stationary matmuls vs Wv (+bv) into V' (bf16) with a ones
           column per head (softmax row-sums for free in the U matmul), then
           Q^T/K^T = W-stationary matmuls vs x^T (+bias) into bf16. Wo is
           staged early (during Q/K) into bf16 so phase 3 never waits on DMA.
  phase 2: per head pair (even head on partitions 0:64, odd on 64:128 —
           adjacent K=64 matmuls land in different PE row-groups and run
           concurrently). The scores PSUM is split by q-half: sp_n [128, S]
           holds q-chunk n for both heads; each half is exp'd by a separate
           ACT instruction into persistent bf16 pt buffers. scores(kk+1)
           issue as soon as exp_n(kk) completes — the ACT engine paces the
           loop at ~2us/kk with the PE's ~1.2us/kk fully hidden.
           U'^T[65, q] accumulates over k-tiles (row 64 = denominator r).
           U rows 0:64 go (unnormalized, bf16) into merged^T. Each pair's
           normalization is pipelined right behind its evacuation: r rows ->
           DRAM -> re-read scattered across 128 partitions (wide DVE
           reciprocal, ~0.25us) -> DRAM -> partition-broadcast -> in-place
           mul. Each pair's DMA chain lives on one ring (sync/gpsimd
           alternating) so chains order internally but overlap across pairs.
  phase 3: out = merged^T.T @ Wo + bo, all bf16 operands. r=0..6
           accumulation for q-groups 0..3 is issued first so the PE
           front-runs while pair 7's normalization drains; ps tiles are
           [128, 2*512] with n innermost so consecutive accumulating matmuls
           alternate PSUM banks (same-bank back-to-back runs at half rate).
"""

import numpy as np

B = 8
S = 1024
H = 1024
NH = 16
D = 64
P = 128
NT = 8          # number of 128-tiles along S or H
SCALE = 0.125   # 1/sqrt(64)

_CACHE = {}


def _build():
    import concourse.bacc as bacc
    import concourse.mybir as mybir
    import concourse.tile as tile
    from concourse.masks import make_identity
    from contextlib import ExitStack

    F32 = mybir.dt.float32
    F32R = mybir.dt.float32r
    BF16 = mybir.dt.bfloat16
    EXP = mybir.ActivationFunctionType.Exp

    nc = bacc.Bacc(None)
    x = nc.dram_tensor("x", [S, H], F32, kind="ExternalInput")
    wq = nc.dram_tensor("Wq", [H, H], F32, kind="ExternalInput")
    wk = nc.dram_tensor("Wk", [H, H], F32, kind="ExternalInput")
    wv = nc.dram_tensor("Wv", [H, H], F32, kind="ExternalInput")
    wo = nc.dram_tensor("Wo", [H, H], F32, kind="ExternalInput")
    bq = nc.dram_tensor("bq", [H], F32, kind="ExternalInput")
    bk = nc.dram_tensor("bk", [H], F32, kind="ExternalInput")
    bv = nc.dram_tensor("bv", [H], F32, kind="ExternalInput")
    bo = nc.dram_tensor("bo", [H], F32, kind="ExternalInput")
    out = nc.dram_tensor("out", [S, H], F32, kind="ExternalOutput")
    rscr = nc.dram_tensor("rscr", [NH, S], F32)   # scratch for r and 1/r

    with tile.TileContext(nc) as tc, ExitStack() as ctx:
        pers = ctx.enter_context(tc.tile_pool(name="pers", bufs=1))
        wpool = ctx.enter_context(tc.tile_pool(name="wpool", bufs=2))
        small = ctx.enter_context(tc.tile_pool(name="small", bufs=1))

        qt = pers.tile([P, NT, S], BF16, name="qt")
        kt = pers.tile([P, NT, S], BF16, name="kt")
        vp = pers.tile([P, NT, NH * (D + 1)], F32R, name="vp")

        bq_sb = small.tile([P, NT], F32, name="bq_sb")
        bk_sb = small.tile([P, NT], F32, name="bk_sb")
        nc.scalar.dma_start(bq_sb[:], bq.rearrange("(r p) -> p r", p=P))
        nc.scalar.dma_start(bk_sb[:], bk.rearrange("(r p) -> p r", p=P))
        bv_bc = small.tile([P, H], F32, name="bv_bc")
        nc.scalar.dma_start(bv_bc[:], bv[None, :].to_broadcast((P, H)))
        ones16 = small.tile([P, NH], F32, name="ones16")
        nc.vector.memset(ones16[:], 1.0)

        # ---- phase 0: x -> xT (xT lives only through phase 1) ----
        xtp = ctx.enter_context(tc.tile_pool(name="xtp", bufs=1))
        xT = xtp.tile([P, NT, S], F32R, name="xT")
        with tc.tile_pool(name="xstage", bufs=4) as xstage, \
             tc.tile_pool(name="idpool", bufs=1) as idpool, \
             tc.tile_pool(name="tpsum", bufs=4, space="PSUM") as tpsum:
            identity = idpool.tile([P, P], F32, name="identity")
            make_identity(nc, identity[:])
            for st in range(NT):
                xs = xstage.tile([P, H], F32, tag="xs", name=f"xs{st}")
                (nc.sync if st % 2 == 0 else nc.gpsimd).dma_start(
                    xs[:], x[P * st:P * (st + 1), :])
                for r in range(NT):
                    tp = tpsum.tile([P, P], F32, tag="tp", name=f"tp{st}_{r}")
                    nc.tensor.transpose(tp[:], xs[:, P * r:P * (r + 1)], identity[:])
                    nc.vector.tensor_copy(xT[:, r, P * st:P * (st + 1)], tp[:])

        # ---- phase 1: projections (V first, then Q, K; Wo staged early) ----
        wopool = ctx.enter_context(tc.tile_pool(name="wopool", bufs=2))
        with tc.tile_pool(name="wextra", bufs=2) as wextra, \
             tc.tile_pool(name="wstage", bufs=4) as wstage, \
             tc.tile_pool(name="ppsum", bufs=4, space="PSUM") as ppsum:

            def load_w_half(pool, tg, wsrc, wname, half, dt, ring):
                w_h = pool.tile([P, 4, H], dt, tag=tg, name=f"w_{wname}{half}")
                for c in range(4):
                    stg = wstage.tile([P, H], F32, tag="wst",
                                      name=f"wst_{wname}{half}_{c}")
                    ring[c % len(ring)].dma_start(
                        stg[:],
                        wsrc[512 * half + P * c:512 * half + P * (c + 1), :])
                    nc.vector.tensor_copy(w_h[:, c, :], stg[:])
                return w_h

            r_sg = [nc.sync, nc.gpsimd]
            wv_t = [load_w_half(wpool, "w", wv, "v", h, F32R, r_sg)
                    for h in range(2)]
            wq_t = [load_w_half(wextra, "wx", wq, "q", h, F32R, r_sg)
                    for h in range(2)]

            # V (natural layout, into vp with ones columns)
            for m in range(NT):
                ps = ppsum.tile([P, S], F32, tag="pp", name=f"ppv{m}")
                for kk in range(NT):
                    for n in range(2):
                        nc.tensor.matmul(
                            ps[:, 512 * n:512 * (n + 1)],
                            xT[:, kk, P * m:P * (m + 1)],
                            wv_t[kk // 4][:, kk % 4, 512 * n:512 * (n + 1)],
                            start=(kk == 0), stop=(kk == NT - 1))
                vview = vp[:, m, :].rearrange("p (h d) -> p h d", d=D + 1)
                nc.vector.tensor_add(
                    vview[:, :, 0:D],
                    ps[:].rearrange("p (h d) -> p h d", d=D),
                    bv_bc[:].rearrange("p (h d) -> p h d", d=D))
                nc.vector.tensor_copy(vview[:, :, D:D + 1], ones16[:].unsqueeze(2))

            wk_t = [load_w_half(wpool, "w", wk, "k", h, F32R, r_sg)
                    for h in range(2)]
            wo_t = [load_w_half(wopool, "wo", wo, "o", h, BF16, [nc.scalar])
                    for h in range(2)]

            for wt_l, dst, bias in ((wq_t, qt, bq_sb), (wk_t, kt, bk_sb)):
                for m in range(NT):
                    ps = ppsum.tile([P, S], F32, tag="pp",
                                    name=f"pp{dst.name}{m}")
                    for kk in range(NT):
                        for n in range(2):
                            nc.tensor.matmul(
                                ps[:, 512 * n:512 * (n + 1)],
                                wt_l[kk // 4][:, kk % 4, P * m:P * (m + 1)],
                                xT[:, kk, 512 * n:512 * (n + 1)],
                                start=(kk == 0), stop=(kk == NT - 1))
                    nc.scalar.add(dst[:, m, :], ps[:], bias[:, m:m + 1])

        # ---- phase 2: attention, head pairs, one continuous pipeline ----
        # The (hp, kk) iterations form a single software-pipelined stream:
        # pair p+1's first scores/exps are emitted before pair p's last U
        # matmuls, so the ACT engine never drains at a pair boundary.
        late = ctx.enter_context(tc.tile_pool(name="late", bufs=1))
        mergedT = late.tile([P, NT, S], BF16, name="mergedT")
        # persistent exp buffers [n-half][kk parity]: f32r moving operands
        # sustain full U-matmul rate where bf16 measured ~2x slower here
        ptb = [[late.tile([P, S], F32R, name=f"pt{n}_{par}")
                for par in range(2)] for n in range(2)]
        r4s = late.tile([P, NT, 16], F32, name="r4s")  # per-pair scattered r

        with tc.tile_pool(name="spsum", bufs=1, space="PSUM") as spsum, \
             tc.tile_pool(name="upsum", bufs=1, space="PSUM") as upsum, \
             tc.tile_pool(name="rrpool", bufs=2) as rrpool, \
             tc.tile_pool(name="rbpool", bufs=2) as rbpool:

            ups = {}

            def u_mms(pts, hp, kk):
                up_e, up_o = ups[hp]
                he, ho = 2 * hp, 2 * hp + 1
                for n in range(2):
                    nc.tensor.matmul(
                        up_e[:, 512 * n:512 * (n + 1)],
                        vp[:, kk, (D + 1) * he:(D + 1) * (he + 1)],
                        pts[n][:, 0:512],
                        start=(kk == 0), stop=(kk == NT - 1))
                    nc.tensor.matmul(
                        up_o[:, 512 * n:512 * (n + 1)],
                        vp[:, kk, (D + 1) * ho:(D + 1) * (ho + 1)],
                        pts[n][:, 512:1024],
                        start=(kk == 0), stop=(kk == NT - 1))

            def evac_pair(hp):
                # evacuate unnormalized U^T + r rows; pipeline this pair's
                # normalization (each pair's DMA chain on its own ring)
                up_e, up_o = ups.pop(hp)
                he, ho = 2 * hp, 2 * hp + 1
                nc.vector.tensor_copy(mergedT[0:D, hp, :], up_e[0:D, :])
                nc.vector.tensor_copy(mergedT[D:P, hp, :], up_o[0:D, :])
                eng = nc.sync if hp % 2 == 0 else nc.gpsimd
                for h, up, c0 in ((he, up_e, 0), (ho, up_o, 8)):
                    rrow = rrpool.tile([1, S], F32, tag="rr", name=f"rr{h}")
                    nc.vector.tensor_copy(rrow[:], up[D:D + 1, :])
                    eng.dma_start(rscr[h, :], rrow[:])
                    eng.dma_start(
                        r4s[:, hp, c0:c0 + 8],
                        rscr[h, :].rearrange("(p c) -> p c", p=P))
                nc.vector.reciprocal(r4s[:, hp, :], r4s[:, hp, :])
                rb = rbpool.tile([P, S], F32, tag="rb", name=f"rb{hp}")
                for h, c0, lo, hi in ((he, 0, 0, D), (ho, 8, D, P)):
                    eng.dma_start(
                        rscr[h, :].rearrange("(p c) -> p c", p=P),
                        r4s[:, hp, c0:c0 + 8])
                    eng.dma_start(
                        rb[lo:hi, :],
                        rscr[h, :][None, :].to_broadcast((hi - lo, S)))
                nc.vector.tensor_mul(mergedT[:, hp, :], mergedT[:, hp, :],
                                     rb[:])

            prev = None
            for hp in range(NH // 2):
                ups[hp] = (
                    upsum.tile([D + 1, S], F32, tag="upe", name=f"up{2 * hp}"),
                    upsum.tile([D + 1, S], F32, tag="upo",
                               name=f"up{2 * hp + 1}"))
                for kk in range(NT):
                    pts = []
                    for n in range(2):
                        # q-chunk n for both heads: cols 0:512 even head,
                        # 512:1024 odd head (separate 2-bank PSUM per n so
                        # each exp only serializes against its own half)
                        sp = spsum.tile([P, S], F32, tag=f"sp{n}",
                                        name=f"sp{hp}_{kk}_{n}")
                        nc.tensor.matmul(
                            sp[:, 0:512],
                            kt[0:D, hp, P * kk:P * (kk + 1)],
                            qt[0:D, hp, 512 * n:512 * (n + 1)],
                            start=True, stop=True)
                        nc.tensor.matmul(
                            sp[:, 512:1024],
                            kt[D:P, hp, P * kk:P * (kk + 1)],
                            qt[D:P, hp, 512 * n:512 * (n + 1)],
                            start=True, stop=True)
                        pt = ptb[n][kk % 2]
                        nc.scalar.activation(pt[:], sp[:], EXP, scale=SCALE)
                        pts.append(pt)
                    if prev is not None:
                        u_mms(*prev)
                        if prev[2] == NT - 1:
                            evac_pair(prev[1])
                    prev = (pts, hp, kk)
            u_mms(*prev)
            evac_pair(prev[1])

        # ---- phase 3: output projection (all bf16 operands) ----
        # pair 7's normalization drains on DVE/DMA while the PE front-runs
        # r=0..6 for q-groups 0..3; only the r=7 matmuls wait on it.
        with tc.tile_pool(name="opsum", bufs=4, space="PSUM") as opsum, \
             tc.tile_pool(name="ostage", bufs=4) as ostage, \
             tc.tile_pool(name="bopool", bufs=1) as bopool:
            bo_bc = bopool.tile([P, H], F32, name="bo_bc")
            nc.scalar.dma_start(bo_bc[:], bo[None, :].to_broadcast((P, H)))

            def op_mm(ps, q, r, n):
                nc.tensor.matmul(
                    ps[:, 512 * n:512 * (n + 1)],
                    mergedT[:, r, P * q:P * (q + 1)],
                    wo_t[r // 4][:, r % 4, 512 * n:512 * (n + 1)],
                    start=(r == 0), stop=(r == NT - 1))

            def op_evac(ps, q):
                os_t = ostage.tile([P, H], F32, tag="os", name=f"os{q}")
                nc.vector.tensor_add(os_t[:], ps[:], bo_bc[:])
                nc.sync.dma_start(out[P * q:P * (q + 1), :], os_t[:])

            ps_head = []
            for q in range(4):
                ps = opsum.tile([P, S], F32, tag="op", name=f"op{q}")
                ps_head.append(ps)
                for r in range(NT - 1):
                    for n in range(2):
                        op_mm(ps, q, r, n)
            for q in range(4):
                for n in range(2):
                    op_mm(ps_head[q], q, NT - 1, n)
                op_evac(ps_head[q], q)
            for q in range(4, NT):
                ps = opsum.tile([P, S], F32, tag="op", name=f"op{q}")
                for r in range(NT):
                    for n in range(2):
                        op_mm(ps, q, r, n)
                op_evac(ps, q)

    nc.finalize()
    return nc


def kernel(**inputs):
    from concourse.bass_utils import run_bass_kernel_spmd

    nc = _CACHE.get("nc")
    if nc is None:
        nc = _CACHE["nc"] = _build()

    x = np.ascontiguousarray(np.asarray(inputs["x"], dtype=np.float32))
    common = {k: np.ascontiguousarray(np.asarray(inputs[k], dtype=np.float32))
              for k in ("Wq", "Wk", "Wv", "Wo", "bq", "bk", "bv", "bo")}
    in_maps = [{"x": x[b], **common} for b in range(B)]
    res = run_bass_kernel_spmd(nc, in_maps, list(range(B)))
    return np.stack([res.results[b]["out"] for b in range(B)]).astype(np.float32)
